# revision 1
# baseline (speedup 1.0000x reference)
"""GATv2 (3 layers, self-loops, segment softmax) on 8 Trainium2 NeuronCores.

Strategy (per spec sharding hint): nodes sharded contiguously across 8 cores;
edges routed to the core owning their dst; per core, edges sorted by dst and
grouped into 128-dst blocks x 128-edge chunks; per layer each core computes
xl/xr for its shard (PE), AllGathers the xl table, then runs the edge phase:
indirect-DMA gathers of xl[src]/xr[dst], score computation on DVE/ACT, and a
one-hot matmul (PE) that does the per-block segment reduction of both the
softmax numerator and denominator in one PSUM accumulation.

Self-contained: hardcodes problem shapes; no sibling imports.
"""
import numpy as np

P = 128          # partitions / block size / chunk size
SC = 4           # chunks per superchunk (batched gathers + elementwise)
NEG_SLOPE = 0.2


# ---------------------------------------------------------------- host prep

def prep_edges(src, dst, N, ncores):
    """Route edges to dst-owning cores, sort by dst, pack into block/chunk slots.

    Returns (Nshard, nblk, NSC, ids) where ids is int32
    [ncores, nblk, NSC, P, 3*SC]: cols [0:SC] global src id, [SC:2*SC] local
    dst id, [2*SC:3*SC] float32-bitcast block-relative dst (1e6 sentinel pad).
    """
    Nshard = ((N + ncores * P - 1) // (ncores * P)) * P
    nblk = Nshard // P
    core = dst // Nshard
    percore = []
    maxch = 1
    for c in range(ncores):
        m = core == c
        s = src[m]
        dl = (dst[m] - c * Nshard).astype(np.int64)
        o = np.argsort(dl, kind='stable')
        s, dl = s[o], dl[o]
        blk = dl // P
        counts = np.bincount(blk, minlength=nblk)
        maxch = max(maxch, int(np.max((counts + P - 1) // P)))
        percore.append((s, dl, blk, counts))
    NSC = (maxch + SC - 1) // SC
    CH = NSC * SC
    ids = np.zeros((ncores, nblk, CH, P, 2), np.int32)
    dst16 = np.zeros((ncores, nblk, CH * P), np.int16)   # block-local dst for dma_gather
    sentinel = np.float32(1e6).view(np.int32)
    ids[:, :, :, :, 1] = sentinel
    for c in range(ncores):
        s, dl, blk, counts = percore[c]
        starts = np.zeros(nblk, np.int64)
        starts[1:] = np.cumsum(counts)[:-1]
        pos = np.arange(len(s)) - starts[blk]        # rank within block
        ch = pos // P
        p = pos % P
        ids[c, blk, ch, p, 0] = s
        ids[c, blk, ch, p, 1] = (dl - blk * P).astype(np.float32).view(np.int32)
        dst16[c, blk, ch * P + p] = dl
    # ids: [nblk, CH, P, 2] -> [nblk, NSC, P, 2*SC] with cols grouped (src*SC, rel*SC)
    ids = ids.reshape(ncores, nblk, NSC, SC, P, 2)
    ids = ids.transpose(0, 1, 2, 4, 5, 3)            # [c, nblk, NSC, P, 2, SC]
    ids = ids.reshape(ncores, nblk, NSC, P, 2 * SC)
    # dst16: wrap for dma_gather: idx i read from [i % 16, i // 16], x8 partitions
    n = CH * P
    dst16 = dst16.reshape(ncores, nblk, n // 16, 16).transpose(0, 1, 3, 2)  # [c,b,16,n/16]
    dst16 = np.tile(dst16, (1, 1, 8, 1))             # [c, nblk, 128, n/16]
    return Nshard, nblk, NSC, np.ascontiguousarray(ids), np.ascontiguousarray(dst16)


# ---------------------------------------------------------------- bass build

def build_program(ncores, Nshard, nblk, NSC, dims_in, H, C, use_collective=True):
    import concourse.bass as bass
    import concourse.mybir as mybir
    from concourse import bacc
    from concourse.tile import TileContext

    D = H * C
    W = D + H
    L = len(dims_in)
    CH = NSC * SC
    Np = Nshard * ncores
    f32, i32 = mybir.dt.float32, mybir.dt.int32
    AF = mybir.ActivationFunctionType
    OP = mybir.AluOpType

    i16 = mybir.dt.int16
    from concourse import library_config

    nc = bacc.Bacc()
    x0 = nc.declare_dram_parameter("x0", [Nshard, dims_in[0]], f32, isOutput=False)
    ids = nc.declare_dram_parameter("ids", [nblk, NSC, P, 2 * SC], i32, isOutput=False)
    dst16 = nc.declare_dram_parameter("dst16", [nblk, P, CH * P // 16], i16, isOutput=False)
    wparams = []
    for l in range(L):
        din = dims_in[l]
        wparams.append((
            nc.declare_dram_parameter(f"Wl{l}", [din, D], f32, isOutput=False),
            nc.declare_dram_parameter(f"Wr{l}", [din, P], f32, isOutput=False),
            nc.declare_dram_parameter(f"attb{l}", [P, D], f32, isOutput=False),
            nc.declare_dram_parameter(f"biasb{l}", [P, D], f32, isOutput=False),
        ))
    ident_in = nc.declare_dram_parameter("ident", [P, P], f32, isOutput=False)
    iota_in = nc.declare_dram_parameter("iota", [P, P], f32, isOutput=False)
    y = nc.declare_dram_parameter("y", [Nshard, D], f32, isOutput=True)

    xl_sh = nc.dram_tensor("xl_sh", [Nshard, D], f32)
    xl_full = nc.dram_tensor("xl_full", [Np, D], f32, addr_space="Shared")
    xr_tab = nc.dram_tensor("xr_tab", [Nshard, P], f32)   # rows padded to 512B for dma_gather
    xmid = [nc.dram_tensor(f"xmid{i}", [Nshard, D], f32) for i in range(L - 1)]

    with TileContext(nc) as tc:
        with (
            tc.tile_pool(name="const", bufs=1) as cp,
            tc.tile_pool(name="ab", bufs=3) as abp,
            tc.tile_pool(name="abps", bufs=2, space="PSUM") as abps,
            tc.tile_pool(name="edge", bufs=3) as ep,
            tc.tile_pool(name="blkps", bufs=2, space="PSUM") as blkps,
            tc.tile_pool(name="fin", bufs=2) as fp,
        ):
            nc.gpsimd.load_library(library_config.mlp)
            ident = cp.tile([P, P], f32, tag="ident")
            nc.sync.dma_start(out=ident[:], in_=ident_in[:])
            iota = cp.tile([P, P], f32, tag="iota")
            nc.sync.dma_start(out=iota[:], in_=iota_in[:])
            wts = []
            for l in range(L):
                din = dims_in[l]
                wl = cp.tile([din, D], f32, tag=f"wl{l}")
                nc.sync.dma_start(out=wl[:], in_=wparams[l][0][:])
                wr = cp.tile([din, P], f32, tag=f"wr{l}")
                nc.sync.dma_start(out=wr[:], in_=wparams[l][1][:])
                attb = cp.tile([P, D], f32, tag=f"attb{l}")
                nc.sync.dma_start(out=attb[:], in_=wparams[l][2][:])
                biasb = cp.tile([P, D], f32, tag=f"biasb{l}")
                nc.sync.dma_start(out=biasb[:], in_=wparams[l][3][:])
                wts.append((wl, wr, attb, biasb))

            for l in range(L):
                din = dims_in[l]
                x_cur = x0 if l == 0 else xmid[l - 1]
                x_out = y if l == L - 1 else xmid[l]
                wl, wr, attb, biasb = wts[l]

                # ---- node transform: xl_sh = x @ Wl, xr_tab = x @ Wr
                for t in range(nblk):
                    xc = abp.tile([P, din], f32, tag="xc")
                    nc.sync.dma_start(out=xc[:], in_=x_cur[t * P:(t + 1) * P, :])
                    xt_ps = abps.tile([din, P], f32, tag="xtps")
                    nc.tensor.transpose(out=xt_ps[:], in_=xc[:], identity=ident[:])
                    xt = abp.tile([din, P], f32, tag="xt")
                    nc.scalar.activation(out=xt[:], in_=xt_ps[:], func=AF.Copy)
                    mml = abps.tile([P, D], f32, tag="mml")
                    nc.tensor.matmul(out=mml[:], lhsT=xt[:], rhs=wl[:], start=True, stop=True)
                    mmr = abps.tile([P, P], f32, tag="mmr")
                    nc.tensor.matmul(out=mmr[:], lhsT=xt[:], rhs=wr[:], start=True, stop=True)
                    sxl = abp.tile([P, D], f32, tag="sxl")
                    nc.vector.tensor_copy(out=sxl[:], in_=mml[:])
                    nc.sync.dma_start(out=xl_sh[t * P:(t + 1) * P, :], in_=sxl[:])
                    sxr = abp.tile([P, P], f32, tag="sxr")
                    nc.scalar.activation(out=sxr[:], in_=mmr[:], func=AF.Copy)
                    nc.sync.dma_start(out=xr_tab[t * P:(t + 1) * P, :], in_=sxr[:])

                # ---- gather table for xl across all cores
                if use_collective:
                    nc.gpsimd.collective_compute(
                        "AllGather", OP.bypass,
                        replica_groups=[list(range(ncores))],
                        ins=[xl_sh[:]], outs=[xl_full[:]],
                    )
                else:
                    nc.sync.dma_start(out=xl_full[:], in_=xl_sh[:])

                # ---- edge phase
                for blk in range(nblk):
                    ps = blkps.tile([P, W], f32, tag="ps")
                    # dma_gather per superchunk (512 idxs; SWDGE ring holds 1024 descs)
                    i16t = ep.tile([P, CH * P // 16], i16, tag="i16t")
                    nc.sync.dma_start(out=i16t[:], in_=dst16[blk, :, :])
                    xr_b = ep.tile([P, CH * P], f32, tag="xrb")
                    xr_v = xr_b[:].rearrange("p (c e) -> p c e", c=CH)
                    nsc_idx = SC * P // 16   # idx columns per superchunk
                    for sc in range(NSC):
                        nc.gpsimd.dma_gather(
                            out_ap=xr_v[:, sc * SC:(sc + 1) * SC, :],
                            in_ap=xr_tab[:],
                            idxs_ap=i16t[:, sc * nsc_idx:(sc + 1) * nsc_idx],
                            num_idxs=SC * P, num_idxs_reg=SC * P, elem_size=P)
                    for sc in range(NSC):
                        idst = ep.tile([P, 2 * SC], i32, tag="idst")
                        nc.sync.dma_start(out=idst[:], in_=ids[blk, sc, :, :])
                        xl_s = ep.tile([P, SC * D], f32, tag="xls")
                        for k in range(SC):
                            # HW indirect DMA: one gathered row per partition per call
                            nc.gpsimd.indirect_dma_start(
                                out=xl_s[:, k * D:(k + 1) * D], out_offset=None, in_=xl_full[:],
                                in_offset=bass.IndirectOffsetOnAxis(ap=idst[:, k:k + 1], axis=0))
                        g = ep.tile([P, SC * D], f32, tag="g")
                        nc.vector.tensor_tensor(
                            out=g[:].rearrange("p (s d) -> p s d", s=SC),
                            in0=xl_s[:].rearrange("p (s d) -> p s d", s=SC),
                            in1=xr_v[:, sc * SC:(sc + 1) * SC, 0:D],
                            op=OP.add)
                        gl = ep.tile([P, SC * D], f32, tag="gl")
                        # leaky_relu(g) = max(0.2*g, g) in one fused DVE op
                        nc.vector.scalar_tensor_tensor(
                            out=gl[:], in0=g[:], scalar=NEG_SLOPE, in1=g[:],
                            op0=OP.mult, op1=OP.max)
                        ge = ep.tile([P, SC * D], f32, tag="ge")
                        nc.vector.tensor_tensor(
                            out=ge[:].rearrange("p (s d) -> p s d", s=SC),
                            in0=gl[:].rearrange("p (s d) -> p s d", s=SC),
                            in1=attb[:].unsqueeze(1).to_broadcast([P, SC, D]),
                            op=OP.mult)
                        e = ep.tile([P, SC * H], f32, tag="e")
                        nc.vector.tensor_reduce(
                            out=e[:].rearrange("p (s h) -> p s h", s=SC),
                            in_=ge[:].rearrange("p (s h c) -> p s h c", s=SC, h=H),
                            axis=mybir.AxisListType.X, op=OP.add)
                        vals = ep.tile([P, SC * W], f32, tag="vals")
                        vals_v = vals[:].rearrange("p (s w) -> p s w", s=SC)
                        nc.scalar.activation(
                            out=vals_v[:, :, D:W],
                            in_=e[:].rearrange("p (s h) -> p s h", s=SC),
                            func=AF.Exp)
                        nc.vector.tensor_tensor(
                            out=vals_v[:, :, 0:D].rearrange("p s (h c) -> p s h c", h=H),
                            in0=xl_s[:].rearrange("p (s h c) -> p s h c", s=SC, h=H),
                            in1=vals_v[:, :, D:W].unsqueeze(3).to_broadcast([P, SC, H, C]),
                            op=OP.mult)
                        ot = ep.tile([P, SC * P], f32, tag="ot")
                        nc.vector.tensor_tensor(
                            out=ot[:].rearrange("p (s q) -> p s q", s=SC),
                            in0=idst[:, SC:2 * SC].bitcast(f32).unsqueeze(2).to_broadcast([P, SC, P]),
                            in1=iota[:].unsqueeze(1).to_broadcast([P, SC, P]),
                            op=OP.is_equal)
                        for k in range(SC):
                            ch = sc * SC + k
                            nc.tensor.matmul(
                                out=ps[:], lhsT=ot[:, k * P:(k + 1) * P],
                                rhs=vals[:, k * W:(k + 1) * W],
                                start=(ch == 0), stop=(ch == CH - 1))
                    # ---- finalize block: div by denom, +bias, elu, store
                    den = fp.tile([P, H], f32, tag="den")
                    nc.vector.tensor_scalar(
                        out=den[:], in0=ps[:, D:W], scalar1=1e-30, scalar2=None, op0=OP.max)
                    r = fp.tile([P, H], f32, tag="r")
                    nc.vector.reciprocal(out=r[:], in_=den[:])
                    o = fp.tile([P, D], f32, tag="o")
                    nc.vector.tensor_tensor(
                        out=o[:].rearrange("p (h c) -> p h c", h=H),
                        in0=ps[:, 0:D].rearrange("p (h c) -> p h c", h=H),
                        in1=r[:].unsqueeze(2).to_broadcast([P, H, C]),
                        op=OP.mult)
                    nc.vector.tensor_tensor(out=o[:], in0=o[:], in1=biasb[:], op=OP.add)
                    t1 = fp.tile([P, D], f32, tag="t1")
                    nc.vector.tensor_scalar(
                        out=t1[:], in0=o[:], scalar1=0.0, scalar2=None, op0=OP.min)
                    nc.scalar.activation(out=t1[:], in_=t1[:], func=AF.Exp)
                    nc.vector.tensor_scalar(
                        out=t1[:], in0=t1[:], scalar1=-1.0, scalar2=None, op0=OP.add)
                    nc.vector.tensor_tensor(out=o[:], in0=o[:], in1=t1[:], op=OP.max)
                    nc.sync.dma_start(out=x_out[blk * P:(blk + 1) * P, :], in_=o[:])
    nc.compile()
    return nc


# ---------------------------------------------------------------- entry

def make_inmaps(inputs, ncores):
    x = np.asarray(inputs['x'], np.float32)
    ei = np.asarray(inputs['edge_index'], np.int32)
    N, F = x.shape
    H, C = np.asarray(inputs['att0']).shape
    D = H * C
    L = 3
    loops = np.arange(N, dtype=np.int32)
    src = np.concatenate([ei[0], loops])
    dst = np.concatenate([ei[1], loops])
    Nshard, nblk, NSC, ids, dst16 = prep_edges(src, dst, N, ncores)
    xp = np.zeros((Nshard * ncores, F), np.float32)
    xp[:N] = x
    iota = np.broadcast_to(np.arange(P, dtype=np.float32), (P, P)).copy()
    ident = np.eye(P, dtype=np.float32)
    dims_in = [F] + [D] * (L - 1)
    base = {"ident": ident, "iota": iota}
    for l in range(L):
        base[f"Wl{l}"] = np.ascontiguousarray(np.asarray(inputs[f'Wl{l}'], np.float32))
        wr = np.asarray(inputs[f'Wr{l}'], np.float32)
        base[f"Wr{l}"] = np.concatenate([wr, np.zeros((wr.shape[0], P - D), np.float32)], 1)
        att = np.asarray(inputs[f'att{l}'], np.float32).reshape(1, D)
        base[f"attb{l}"] = np.broadcast_to(att, (P, D)).copy()
        b = np.asarray(inputs[f'b{l}'], np.float32).reshape(1, D)
        base[f"biasb{l}"] = np.broadcast_to(b, (P, D)).copy()
    in_maps = []
    for c in range(ncores):
        m = dict(base)
        m["x0"] = np.ascontiguousarray(xp[c * Nshard:(c + 1) * Nshard])
        m["ids"] = np.ascontiguousarray(ids[c])
        m["dst16"] = np.ascontiguousarray(dst16[c])
        in_maps.append(m)
    return in_maps, Nshard, nblk, NSC, dims_in, H, C, N, D


def kernel(**inputs):
    from concourse.bass_utils import run_bass_kernel_spmd
    ncores = 8
    in_maps, Nshard, nblk, NSC, dims_in, H, C, N, D = make_inmaps(inputs, ncores)
    nc = build_program(ncores, Nshard, nblk, NSC, dims_in, H, C, use_collective=True)
    res = run_bass_kernel_spmd(nc, in_maps, list(range(ncores)))
    out = np.concatenate([res.results[c]["y"] for c in range(ncores)], axis=0)
    return out[:N].astype(np.float32)


if __name__ == "__main__":
    pass



# revision 14
# speedup vs baseline: 1.1298x; 1.1298x over previous
"""GATv2 (3 layers, self-loops, segment softmax) on 8 Trainium2 NeuronCores.

v2 design (vs v1 baseline):
- bf16 activations/weights end-to-end (PSUM accumulation f32); 2x DVE and
  4x PE throughput, half the DMA/collective bytes.
- Big dma_gather calls (single_packet=False) replace v1's per-chunk
  indirect_dma_start + small dma_gathers: ~125 Pool calls/layer instead of
  ~1470 (Pool/SWDGE was 9.7ms of the 12.1ms v1 span).
- Node tables padded to 256B rows ([*, 128] bf16) to satisfy dma_gather's
  elem%256B constraint; int16 gather indices via 4 windows of 25088 rows.
- Edges packed into groups of G=4 dst blocks; per (group, window, block) a
  static column budget (max over cores) keeps the program SPMD-uniform.
- Per-edge score pipeline on DVE/ACT in bf16; segment softmax numerator and
  denominator via one-hot matmul accumulation in PSUM (bf16 operands).

Sharding: nodes contiguously across 8 cores (Nshard=12544); edges routed to
dst-owning core; per layer: own-shard transform -> AllGather xl (padded
bf16) -> edge phase.

Self-contained: hardcodes problem shapes; no sibling imports.
"""
import numpy as np
import ml_dtypes

P = 128
N_NODES = 100000
F_IN = 128
H, C = 8, 10
D = H * C              # 80
W = D + H              # 88 (numerator cols + denom cols)
L = 3
NCORES = 8
NSHARD = 12544         # ceil(100000 / (8*128)) * 128
NBLK = 98              # NSHARD / P
NP = NSHARD * NCORES   # 100352
G = 4                  # dst blocks per edge group
NGRP = 25              # 24 full groups + 1 group with 2 real + 2 pad blocks
NW = 4                 # gather windows (int16 index range)
WIN = NP // NW         # 25088 rows per window
MAXIDX = 8192          # max idxs per dma_gather call (ring-limited, spFalse)
NEG_SLOPE = 0.2
BF16 = ml_dtypes.bfloat16


# ---------------------------------------------------------------- host prep

def prep_edges(src, dst):
    """Route edges to dst cores; build per-core gather/one-hot metadata.

    Slot space: per group g (G=4 dst blocks), columns of 128 edge slots.
    Column layout per group: for w in 0..3: for b in 0..G-1: budget[g][w][b]
    columns (static = max over cores, so the compiled program is SPMD).

    Returns (layout, percore) where layout has the static column maps and
    percore has per-core index/reld arrays.
    """
    core = dst // NSHARD
    # per-core sorted edge lists
    edges = []
    for c in range(NCORES):
        m = core == c
        s = src[m].astype(np.int64)
        dl = (dst[m] - c * NSHARD).astype(np.int64)
        b = dl // P                      # dst block 0..97
        g = b // G                       # group 0..24
        bi = b % G                       # block-in-group
        w = s // WIN                     # window 0..3
        edges.append((s, dl, g, bi, w))

    # counts[c, g, w, bi]
    counts = np.zeros((NCORES, NGRP, NW, G), np.int64)
    for c in range(NCORES):
        s, dl, g, bi, w = edges[c]
        np.add.at(counts, (c, g, w, bi), 1)
    maxcnt = counts.max(axis=0)                      # [NGRP, NW, G]
    budget = -(-maxcnt // P)                         # cols per (g, w, b)

    # static column layout per group
    colbase = np.zeros((NGRP, NW, G), np.int64)      # start col of segment
    ncols_g = np.zeros(NGRP, np.int64)
    blk_of_col = []                                  # per g: [ncols] block idx
    w_of_col = []
    for g in range(NGRP):
        cur = 0
        bc, wc = [], []
        for w in range(NW):
            for b in range(G):
                colbase[g, w, b] = cur
                nb = int(budget[g, w, b])
                cur += nb
                bc += [b] * nb
                wc += [w] * nb
        ncols_g[g] = cur
        blk_of_col.append(np.array(bc, np.int64))
        w_of_col.append(np.array(wc, np.int64))

    # per (g, w): idx count and call split
    nidx_gw = (budget.sum(axis=2) * P).astype(np.int64)   # [NGRP, NW]
    # gather calls: (g, w, idx_off_in_window, n, out_colbase)
    calls = []
    for g in range(NGRP):
        for w in range(NW):
            n = int(nidx_gw[g, w])
            off = 0
            while off < n:
                k = min(MAXIDX, n - off)
                calls.append((g, w, off, k, int(colbase[g, w, 0]) + off // P))
                off += k

    # per-core slot-ordered values
    percore = []
    for c in range(NCORES):
        s, dl, g, bi, w = edges[c]
        tot = int(ncols_g.sum())
        # global slot id = (cum cols before g + local col)*128 + partition
        gc0 = np.concatenate([[0], np.cumsum(ncols_g)])[:-1]       # per g
        # rank within (g, w, bi)
        key = (g * NW + w) * G + bi
        order = np.argsort(key, kind='stable')
        ks = key[order]
        rank = np.arange(len(ks)) - np.searchsorted(ks, ks, side='left')
        # build explicit arrays
        xl_idx = np.zeros((tot, P), np.int64)          # [col, p] -> window idx
        xr_idx = np.zeros((tot, P), np.int64)
        reld = np.full((tot, P), 1e6, np.float32)
        gcol = gc0[g[order]] + colbase[g[order], w[order], bi[order]] + rank // P
        part = rank % P
        so, dlo = s[order], dl[order]
        xl_idx[gcol, part] = so - w[order] * WIN
        xr_idx[gcol, part] = dlo
        reld[gcol, part] = dlo - (g[order] * G + bi[order]) * P
        percore.append((xl_idx, xr_idx, reld))

    layout = dict(budget=budget, colbase=colbase, ncols_g=ncols_g,
                  blk_of_col=blk_of_col, w_of_col=w_of_col,
                  nidx_gw=nidx_gw, calls=calls,
                  gc0=np.concatenate([[0], np.cumsum(ncols_g)])[:-1])
    return layout, percore


def wrap_i16(vals):
    """Wrap a flat idx array [n] (n % 16 == 0) into the dma_gather layout
    [128, n/16]: idx j -> [j%16 (+16k replicas), j//16]."""
    n = len(vals)
    w = vals.reshape(n // 16, 16).T                  # [16, n/16]
    return np.tile(w, (8, 1)).astype(np.int16)       # [128, n/16]


def build_idx_tensors(layout, percore):
    """Per-core DRAM tensors: xlidx/xridx [128, TOTI] i16 (call-major,
    wrapped per call), reld [128, TOTC] bf16 (slot-major)."""
    calls = layout['calls']
    ncols_g = layout['ncols_g']
    gc0 = layout['gc0']
    TOTC = int(ncols_g.sum())
    TOTI = sum(k for (_, _, _, k, _) in calls) // 16
    out = []
    for c in range(NCORES):
        xl_idx, xr_idx, reld = percore[c]            # [TOTC, P] slot-major
        xl_flat = np.zeros((P, TOTI), np.int16)
        xr_flat = np.zeros((P, TOTI), np.int16)
        io = 0
        for (g, w, off, k, ocb) in calls:
            c0 = gc0[g] + ocb                         # global out col base
            ncol = k // P
            xlv = xl_idx[c0:c0 + ncol].reshape(-1)    # [k] slot order
            xrv = xr_idx[c0:c0 + ncol].reshape(-1)
            xl_flat[:, io:io + k // 16] = wrap_i16(xlv)
            xr_flat[:, io:io + k // 16] = wrap_i16(xrv)
            io += k // 16
        reldt = np.ascontiguousarray(reld.T.astype(BF16))   # [P, TOTC]
        out.append((xl_flat, xr_flat, reldt))
    return out, TOTC, TOTI


# ---------------------------------------------------------------- bass build

def build_program(layout, TOTC, TOTI, debug=False):
    import concourse.bass as bass
    import concourse.mybir as mybir
    from concourse import bacc, library_config
    from concourse.tile import TileContext

    f32 = mybir.dt.float32
    i16 = mybir.dt.int16
    bf16 = mybir.dt.bfloat16
    AF = mybir.ActivationFunctionType
    OP = mybir.AluOpType

    calls = layout['calls']
    ncols_g = layout['ncols_g']
    gc0 = layout['gc0']
    blk_of_col = layout['blk_of_col']
    dims_in = [F_IN, D, D]

    nc = bacc.Bacc()
    x0 = nc.declare_dram_parameter("x0", [NSHARD, F_IN], bf16, isOutput=False)
    xlidx = nc.declare_dram_parameter("xlidx", [P, TOTI], i16, isOutput=False)
    xridx = nc.declare_dram_parameter("xridx", [P, TOTI], i16, isOutput=False)
    reldp = nc.declare_dram_parameter("reld", [P, TOTC], bf16, isOutput=False)
    wp = []
    for l in range(L):
        din = dims_in[l]
        wp.append((
            nc.declare_dram_parameter(f"Wl{l}", [din, D], bf16, isOutput=False),
            nc.declare_dram_parameter(f"Wr{l}", [din, D], bf16, isOutput=False),
            nc.declare_dram_parameter(f"attb{l}", [P, D], bf16, isOutput=False),
            nc.declare_dram_parameter(f"biasb{l}", [P, D], f32, isOutput=False),
        ))
    ident_in = nc.declare_dram_parameter("ident", [P, P], bf16, isOutput=False)
    iota_in = nc.declare_dram_parameter("iota", [P, P], bf16, isOutput=False)
    y = nc.declare_dram_parameter("y", [NGRP * G * P, D], f32, isOutput=True)
    if debug:
        dbg_xl = nc.declare_dram_parameter("dbg_xl", [NP, P], bf16, isOutput=True)
        dbg_xr = nc.declare_dram_parameter("dbg_xr", [NSHARD, P], bf16, isOutput=True)
        dbg_y0 = nc.declare_dram_parameter("dbg_y0", [NGRP * G * P, D], bf16, isOutput=True)
        c0g = int(layout['ncols_g'][0])
        dbg_xls = nc.declare_dram_parameter("dbg_xls", [P, c0g * P], bf16, isOutput=True)
        dbg_xrs = nc.declare_dram_parameter("dbg_xrs", [P, c0g * P], bf16, isOutput=True)
        dbg_vals = nc.declare_dram_parameter("dbg_vals", [P, c0g * W], bf16, isOutput=True)
        dbg_ot = nc.declare_dram_parameter("dbg_ot", [P, c0g * P], bf16, isOutput=True)

    # internal DRAM (ping-pong across layers to avoid WAR on gathers)
    xl_sh = [nc.dram_tensor(f"xl_sh{i}", [NSHARD, P], bf16) for i in range(2)]
    xl_full = [nc.dram_tensor(f"xl_full{i}", [NP, P], bf16, addr_space="Shared")
               for i in range(2)]
    xr_tab = [nc.dram_tensor(f"xr_tab{i}", [NSHARD, P], bf16) for i in range(2)]
    xmid = [nc.dram_tensor(f"xmid{i}", [NGRP * G * P, D], bf16) for i in range(2)]

    with TileContext(nc) as tc:
        with (
            tc.tile_pool(name="const", bufs=1) as cp,
            tc.tile_pool(name="tf", bufs=3) as tfp,
            tc.tile_pool(name="tfps", bufs=1, space="PSUM") as tfps,
            tc.tile_pool(name="eidx", bufs=2) as eip,
            tc.tile_pool(name="edge", bufs=2) as ep,
            tc.tile_pool(name="eps", bufs=1, space="PSUM") as eps,
            tc.tile_pool(name="fin", bufs=2) as fp_,
        ):
            nc.gpsimd.load_library(library_config.mlp)
            ident = cp.tile([P, P], bf16, tag="ident")
            nc.sync.dma_start(out=ident[:], in_=ident_in[:])
            iota = cp.tile([P, P], bf16, tag="iota")
            nc.sync.dma_start(out=iota[:], in_=iota_in[:])
            wts = []
            for l in range(L):
                din = dims_in[l]
                wl = cp.tile([din, D], bf16, tag=f"wl{l}")
                nc.sync.dma_start(out=wl[:], in_=wp[l][0][:])
                wr = cp.tile([din, D], bf16, tag=f"wr{l}")
                nc.sync.dma_start(out=wr[:], in_=wp[l][1][:])
                attb = cp.tile([P, D], bf16, tag=f"attb{l}")
                nc.sync.dma_start(out=attb[:], in_=wp[l][2][:])
                biasb = cp.tile([P, D], f32, tag=f"biasb{l}")
                nc.sync.dma_start(out=biasb[:], in_=wp[l][3][:])
                wts.append((wl, wr, attb, biasb))

            for l in range(L):
                din = dims_in[l]
                wl, wr, attb, biasb = wts[l]
                xlS, xlF, xrT = xl_sh[l % 2], xl_full[l % 2], xr_tab[l % 2]
                x_cur = x0 if l == 0 else xmid[(l - 1) % 2]
                x_out = y if l == L - 1 else xmid[l % 2]

                # ---- node transform (own shard, groups of 4 blocks)
                for t0 in range(0, NBLK, 4):
                    gsz = min(4, NBLK - t0)
                    rb = t0 * P
                    xc = tfp.tile([P, gsz * din], bf16, tag="xc")
                    nc.sync.dma_start(
                        out=xc[:].rearrange("p (g d) -> p g d", g=gsz),
                        in_=x_cur[rb:rb + gsz * P, 0:din].rearrange(
                            "(g p) d -> p g d", p=P))
                    xt_ps = tfps.tile([din, gsz * P], bf16, tag="xtps")
                    for k in range(gsz):
                        nc.tensor.transpose(
                            out=xt_ps[:, k * P:(k + 1) * P],
                            in_=xc[:, k * din:(k + 1) * din],
                            identity=ident[:])
                    xt = tfp.tile([din, gsz * P], bf16, tag="xt")
                    nc.scalar.activation(out=xt[:], in_=xt_ps[:], func=AF.Copy)
                    mml = tfps.tile([P, gsz * D], f32, tag="mml")
                    mmr = tfps.tile([P, gsz * D], f32, tag="mmr")
                    for k in range(gsz):
                        nc.tensor.matmul(
                            out=mml[:, k * D:(k + 1) * D],
                            lhsT=xt[:, k * P:(k + 1) * P], rhs=wl[:],
                            start=True, stop=True)
                        nc.tensor.matmul(
                            out=mmr[:, k * D:(k + 1) * D],
                            lhsT=xt[:, k * P:(k + 1) * P], rhs=wr[:],
                            start=True, stop=True)
                    xlo = tfp.tile([P, gsz * P], bf16, tag="xlo")
                    xro = tfp.tile([P, gsz * P], bf16, tag="xro")
                    nc.scalar.activation(
                        out=xlo[:].rearrange("p (g d) -> p g d", g=gsz)[:, :, 0:D],
                        in_=mml[:].rearrange("p (g d) -> p g d", g=gsz),
                        func=AF.Copy)
                    nc.vector.tensor_copy(
                        out=xro[:].rearrange("p (g d) -> p g d", g=gsz)[:, :, 0:D],
                        in_=mmr[:].rearrange("p (g d) -> p g d", g=gsz))
                    nc.sync.dma_start(
                        out=xlS[rb:rb + gsz * P, :].rearrange(
                            "(g p) d -> p g d", p=P),
                        in_=xlo[:].rearrange("p (g d) -> p g d", g=gsz))
                    nc.sync.dma_start(
                        out=xrT[rb:rb + gsz * P, :].rearrange(
                            "(g p) d -> p g d", p=P),
                        in_=xro[:].rearrange("p (g d) -> p g d", g=gsz))

                # ---- AllGather xl (padded bf16 rows)
                nc.gpsimd.collective_compute(
                    "AllGather", OP.bypass,
                    replica_groups=[list(range(NCORES))],
                    ins=[xlS[:]], outs=[xlF[:]],
                )

                if debug and l == 0:
                    nc.sync.dma_start(out=dbg_xl[:], in_=xlF[:])
                    nc.sync.dma_start(out=dbg_xr[:], in_=xrT[:])

                # ---- edge phase (25 groups of 4 blocks)
                gcalls = {}
                io = 0
                for (g_, w_, off_, k_, ocb_) in calls:
                    gcalls.setdefault(g_, []).append((w_, off_, k_, ocb_, io))
                    io += k_ // 16
                for g in range(NGRP):
                    cols = int(ncols_g[g])
                    c0 = int(gc0[g])
                    boc = blk_of_col[g]
                    # idx/reld tiles
                    myc = gcalls.get(g, [])
                    i0 = myc[0][4]
                    itot = sum(k_ // 16 for (_, _, k_, _, _) in myc)
                    xli = eip.tile([P, itot], i16, tag="xli")
                    nc.sync.dma_start(out=xli[:], in_=xlidx[:, i0:i0 + itot])
                    xri = eip.tile([P, itot], i16, tag="xri")
                    nc.sync.dma_start(out=xri[:], in_=xridx[:, i0:i0 + itot])
                    reld = eip.tile([P, cols], bf16, tag="reld")
                    nc.sync.dma_start(out=reld[:], in_=reldp[:, c0:c0 + cols])
                    # gathers
                    xl_s = ep.tile([P, cols * P], bf16, tag="xls")
                    xl_v = xl_s[:].rearrange("p (c e) -> p c e", c=cols)
                    xr_s = ep.tile([P, cols * P], bf16, tag="xrs")
                    xr_v = xr_s[:].rearrange("p (c e) -> p c e", c=cols)
                    for (w_, off_, k_, ocb_, io_) in myc:
                        nc.gpsimd.dma_gather(
                            out_ap=xl_v[:, ocb_:ocb_ + k_ // P, :],
                            in_ap=xlF[w_ * WIN:(w_ + 1) * WIN, :],
                            idxs_ap=xli[:, io_ - i0:io_ - i0 + k_ // 16],
                            num_idxs=k_, num_idxs_reg=k_, elem_size=P,
                            single_packet=False)
                        nc.gpsimd.dma_gather(
                            out_ap=xr_v[:, ocb_:ocb_ + k_ // P, :],
                            in_ap=xrT[:],
                            idxs_ap=xri[:, io_ - i0:io_ - i0 + k_ // 16],
                            num_idxs=k_, num_idxs_reg=k_, elem_size=P,
                            single_packet=False)
                    # scores: g = leaky(xl+xr); u = g*att; e = sum_c u
                    nc.vector.tensor_tensor(
                        out=xr_v[:, :, 0:D], in0=xl_v[:, :, 0:D],
                        in1=xr_v[:, :, 0:D], op=OP.add)
                    nc.scalar.activation(
                        out=xr_v[:, :, 0:D], in_=xr_v[:, :, 0:D],
                        func=AF.Prelu, alpha=NEG_SLOPE)
                    nc.vector.tensor_tensor(
                        out=xr_v[:, :, 0:D], in0=xr_v[:, :, 0:D],
                        in1=attb[:].unsqueeze(1).to_broadcast([P, cols, D]),
                        op=OP.mult)
                    e_t = ep.tile([P, cols * H], f32, tag="e")
                    nc.vector.tensor_reduce(
                        out=e_t[:].rearrange("p (c h) -> p c h", c=cols),
                        in_=xr_v[:, :, 0:D].rearrange(
                            "p c (h k) -> p c h k", h=H),
                        axis=mybir.AxisListType.X, op=OP.add)
                    vals = ep.tile([P, cols * W], bf16, tag="vals")
                    vals_v = vals[:].rearrange("p (c w) -> p c w", c=cols)
                    nc.scalar.activation(
                        out=vals_v[:, :, D:W],
                        in_=e_t[:].rearrange("p (c h) -> p c h", c=cols),
                        func=AF.Exp)
                    nc.vector.tensor_tensor(
                        out=vals_v[:, :, 0:D].rearrange(
                            "p c (h k) -> p c h k", h=H),
                        in0=xl_v[:, :, 0:D].rearrange(
                            "p c (h k) -> p c h k", h=H),
                        in1=vals_v[:, :, D:W].unsqueeze(3).to_broadcast(
                            [P, cols, H, C]),
                        op=OP.mult)
                    ot = ep.tile([P, cols * P], bf16, tag="ot")
                    nc.vector.tensor_tensor(
                        out=ot[:].rearrange("p (c q) -> p c q", c=cols),
                        in0=reld[:].unsqueeze(2).to_broadcast([P, cols, P]),
                        in1=iota[:].unsqueeze(1).to_broadcast([P, cols, P]),
                        op=OP.is_equal)
                    if debug and l == 0 and g == 0:
                        nc.sync.dma_start(out=dbg_xls[:], in_=xl_s[:])
                        nc.sync.dma_start(out=dbg_xrs[:], in_=xr_s[:])
                        nc.sync.dma_start(out=dbg_vals[:], in_=vals[:])
                        nc.sync.dma_start(out=dbg_ot[:], in_=ot[:])
                    # aggregation — one PSUM tile (bank) per dst block:
                    # interleaved accumulation regions within one bank are
                    # broken on HW (later start=True clobbers earlier
                    # regions' partials)
                    psb = []
                    for b_ in range(G):
                        t_ = eps.tile([P, W], f32, tag=f"ps{b_}",
                                      name=f"ps{b_}_{l}_{g}")
                        psb.append(t_)
                    first = [True] * G
                    lastcol = {}
                    for j in range(cols):
                        lastcol[int(boc[j])] = j
                    for b_ in range(G):
                        if b_ not in lastcol:      # pad block: no edges
                            nc.vector.memset(psb[b_][:], 0.0)
                    for j in range(cols):
                        b = int(boc[j])
                        nc.tensor.matmul(
                            out=psb[b][:],
                            lhsT=ot[:, j * P:(j + 1) * P],
                            rhs=vals[:, j * W:(j + 1) * W],
                            start=first[b], stop=(lastcol[b] == j))
                        first[b] = False
                    # finalize
                    den = fp_.tile([P, G * H], f32, tag="den")
                    for b_ in range(G):
                        nc.vector.tensor_scalar(
                            out=den[:, b_ * H:(b_ + 1) * H],
                            in0=psb[b_][:, D:W], scalar1=1e-30, scalar2=None,
                            op0=OP.max)
                    r = fp_.tile([P, G * H], f32, tag="r")
                    nc.vector.reciprocal(out=r[:], in_=den[:])
                    o = fp_.tile([P, G * D], f32, tag="o")
                    o_v = o[:].rearrange("p (g d) -> p g d", g=G)
                    for b_ in range(G):
                        nc.vector.tensor_tensor(
                            out=o[:, b_ * D:(b_ + 1) * D].rearrange(
                                "p (h k) -> p h k", h=H),
                            in0=psb[b_][:, 0:D].rearrange(
                                "p (h k) -> p h k", h=H),
                            in1=r[:, b_ * H:(b_ + 1) * H].unsqueeze(2)
                                .to_broadcast([P, H, C]),
                            op=OP.mult)
                    nc.vector.tensor_tensor(
                        out=o_v[:], in0=o_v[:],
                        in1=biasb[:].unsqueeze(1).to_broadcast([P, G, D]),
                        op=OP.add)
                    t1 = fp_.tile([P, G * D], f32, tag="t1")
                    nc.vector.tensor_scalar(
                        out=t1[:], in0=o[:], scalar1=0.0, scalar2=None,
                        op0=OP.min)
                    nc.scalar.activation(out=t1[:], in_=t1[:], func=AF.Exp)
                    yo = fp_.tile([P, G * D], f32 if l == L - 1 else bf16,
                                  tag="yo")
                    nc.vector.scalar_tensor_tensor(
                        out=yo[:], in0=t1[:], scalar=-1.0, in1=o[:],
                        op0=OP.add, op1=OP.max)
                    nc.sync.dma_start(
                        out=x_out[g * G * P:(g + 1) * G * P, :].rearrange(
                            "(g p) d -> p g d", p=P),
                        in_=yo[:].rearrange("p (g d) -> p g d", g=G))
                if debug and l == 0:
                    nc.sync.dma_start(out=dbg_y0[:], in_=xmid[0][:])
    nc.compile()
    return nc


# ---------------------------------------------------------------- entry

def make_inmaps(inputs):
    x = np.asarray(inputs['x'], np.float32)
    ei = np.asarray(inputs['edge_index'], np.int32)
    loops = np.arange(N_NODES, dtype=np.int64)
    src = np.concatenate([ei[0].astype(np.int64), loops])
    dst = np.concatenate([ei[1].astype(np.int64), loops])
    layout, percore = prep_edges(src, dst)
    idx_tensors, TOTC, TOTI = build_idx_tensors(layout, percore)

    xp = np.zeros((NP, F_IN), np.float32)
    xp[:N_NODES] = x
    xp16 = xp.astype(BF16)
    iota = np.broadcast_to(np.arange(P, dtype=np.float32), (P, P)).astype(BF16)
    ident = np.eye(P, dtype=np.float32).astype(BF16)
    base = {"ident": np.ascontiguousarray(ident),
            "iota": np.ascontiguousarray(iota)}
    for l in range(L):
        wlv = np.asarray(inputs[f'Wl{l}'], np.float32).astype(BF16)
        wrv = np.asarray(inputs[f'Wr{l}'], np.float32).astype(BF16)
        att = np.asarray(inputs[f'att{l}'], np.float32).reshape(1, D)
        b = np.asarray(inputs[f'b{l}'], np.float32).reshape(1, D)
        base[f"Wl{l}"] = np.ascontiguousarray(wlv)
        base[f"Wr{l}"] = np.ascontiguousarray(wrv)
        base[f"attb{l}"] = np.ascontiguousarray(
            np.broadcast_to(att, (P, D)).astype(BF16))
        base[f"biasb{l}"] = np.ascontiguousarray(
            np.broadcast_to(b, (P, D)).astype(np.float32))
    in_maps = []
    for c in range(NCORES):
        xl_flat, xr_flat, reldt = idx_tensors[c]
        m = dict(base)
        m["x0"] = np.ascontiguousarray(xp16[c * NSHARD:(c + 1) * NSHARD])
        m["xlidx"] = np.ascontiguousarray(xl_flat)
        m["xridx"] = np.ascontiguousarray(xr_flat)
        m["reld"] = reldt
        in_maps.append(m)
    return in_maps, layout, TOTC, TOTI


def kernel(**inputs):
    from concourse.bass_utils import run_bass_kernel_spmd
    in_maps, layout, TOTC, TOTI = make_inmaps(inputs)
    nc = build_program(layout, TOTC, TOTI)
    res = run_bass_kernel_spmd(nc, in_maps, list(range(NCORES)))
    out = np.concatenate(
        [np.asarray(res.results[c]["y"])[:NSHARD] for c in range(NCORES)],
        axis=0)
    return out[:N_NODES].astype(np.float32)


if __name__ == "__main__":
    pass


# revision 22
# speedup vs baseline: 1.3728x; 1.2150x over previous
"""GATv2 (3 layers, self-loops, segment softmax) on 8 Trainium2 NeuronCores.

v2 design (vs v1 baseline):
- bf16 activations/weights end-to-end (PSUM accumulation f32); 2x DVE and
  4x PE throughput, half the DMA/collective bytes.
- Big dma_gather calls (single_packet=False) replace v1's per-chunk
  indirect_dma_start + small dma_gathers: ~125 Pool calls/layer instead of
  ~1470 (Pool/SWDGE was 9.7ms of the 12.1ms v1 span).
- Node tables padded to 256B rows ([*, 128] bf16) to satisfy dma_gather's
  elem%256B constraint; int16 gather indices via 4 windows of 25088 rows.
- Edges packed into groups of G=4 dst blocks; per (group, window, block) a
  static column budget (max over cores) keeps the program SPMD-uniform.
- Per-edge score pipeline on DVE/ACT in bf16; segment softmax numerator and
  denominator via one-hot matmul accumulation in PSUM (bf16 operands).

Sharding: nodes contiguously across 8 cores (Nshard=12544); edges routed to
dst-owning core; per layer: own-shard transform -> AllGather xl (padded
bf16) -> edge phase.

Self-contained: hardcodes problem shapes; no sibling imports.
"""
import numpy as np
import ml_dtypes

P = 128
N_NODES = 100000
F_IN = 128
H, C = 8, 10
D = H * C              # 80
W = D + H              # 88 (numerator cols + denom cols)
L = 3
NCORES = 8
NSHARD = 12544         # ceil(100000 / (8*128)) * 128
NBLK = 98              # NSHARD / P
NP = NSHARD * NCORES   # 100352
G = 4                  # dst blocks per edge group
NGRP = 25              # 24 full groups + 1 group with 2 real + 2 pad blocks
NW = 4                 # gather windows (int16 index range)
WIN = NP // NW         # 25088 rows per window
MAXIDX = 8192          # max idxs per dma_gather call (ring-limited, spFalse)
NEG_SLOPE = 0.2
BF16 = ml_dtypes.bfloat16


# ---------------------------------------------------------------- host prep

def prep_edges(src, dst):
    """Route edges to dst cores; build per-core gather/one-hot metadata.

    Slot space: per group g (G=4 dst blocks), columns of 128 edge slots.
    Column layout per group: for w in 0..3: for b in 0..G-1: budget[g][w][b]
    columns (static = max over cores, so the compiled program is SPMD).

    Returns (layout, percore) where layout has the static column maps and
    percore has per-core index/reld arrays.
    """
    core = dst // NSHARD
    # per-core sorted edge lists
    edges = []
    for c in range(NCORES):
        m = core == c
        s = src[m].astype(np.int64)
        dl = (dst[m] - c * NSHARD).astype(np.int64)
        b = dl // P                      # dst block 0..97
        g = b // G                       # group 0..24
        bi = b % G                       # block-in-group
        w = s // WIN                     # window 0..3
        edges.append((s, dl, g, bi, w))

    # counts[c, g, w, bi]
    counts = np.zeros((NCORES, NGRP, NW, G), np.int64)
    for c in range(NCORES):
        s, dl, g, bi, w = edges[c]
        np.add.at(counts, (c, g, w, bi), 1)
    maxcnt = counts.max(axis=0)                      # [NGRP, NW, G]
    budget = -(-maxcnt // P)                         # cols per (g, w, b)

    # static column layout per group
    colbase = np.zeros((NGRP, NW, G), np.int64)      # start col of segment
    ncols_g = np.zeros(NGRP, np.int64)
    blk_of_col = []                                  # per g: [ncols] block idx
    w_of_col = []
    for g in range(NGRP):
        cur = 0
        bc, wc = [], []
        for w in range(NW):
            for b in range(G):
                colbase[g, w, b] = cur
                nb = int(budget[g, w, b])
                cur += nb
                bc += [b] * nb
                wc += [w] * nb
        ncols_g[g] = cur
        blk_of_col.append(np.array(bc, np.int64))
        w_of_col.append(np.array(wc, np.int64))

    # per (g, w): idx count and call split
    nidx_gw = (budget.sum(axis=2) * P).astype(np.int64)   # [NGRP, NW]
    # gather calls: (g, w, idx_off_in_window, n, out_colbase)
    calls = []
    for g in range(NGRP):
        for w in range(NW):
            n = int(nidx_gw[g, w])
            off = 0
            while off < n:
                k = min(MAXIDX, n - off)
                calls.append((g, w, off, k, int(colbase[g, w, 0]) + off // P))
                off += k

    # per-core slot-ordered values
    percore = []
    for c in range(NCORES):
        s, dl, g, bi, w = edges[c]
        tot = int(ncols_g.sum())
        # global slot id = (cum cols before g + local col)*128 + partition
        gc0 = np.concatenate([[0], np.cumsum(ncols_g)])[:-1]       # per g
        # rank within (g, w, bi)
        key = (g * NW + w) * G + bi
        order = np.argsort(key, kind='stable')
        ks = key[order]
        rank = np.arange(len(ks)) - np.searchsorted(ks, ks, side='left')
        # build explicit arrays
        xl_idx = np.zeros((tot, P), np.int64)          # [col, p] -> window idx
        xr_idx = np.zeros((tot, P), np.int64)
        reld = np.full((tot, P), 1e6, np.float32)
        gcol = gc0[g[order]] + colbase[g[order], w[order], bi[order]] + rank // P
        part = rank % P
        so, dlo = s[order], dl[order]
        xl_idx[gcol, part] = so - w[order] * WIN
        xr_idx[gcol, part] = dlo
        reld[gcol, part] = dlo - (g[order] * G + bi[order]) * P
        percore.append((xl_idx, xr_idx, reld))

    layout = dict(budget=budget, colbase=colbase, ncols_g=ncols_g,
                  blk_of_col=blk_of_col, w_of_col=w_of_col,
                  nidx_gw=nidx_gw, calls=calls,
                  gc0=np.concatenate([[0], np.cumsum(ncols_g)])[:-1])
    return layout, percore


def wrap_i16(vals):
    """Wrap a flat idx array [n] (n % 16 == 0) into the dma_gather layout
    [128, n/16]: idx j -> [j%16 (+16k replicas), j//16]."""
    n = len(vals)
    w = vals.reshape(n // 16, 16).T                  # [16, n/16]
    return np.tile(w, (8, 1)).astype(np.int16)       # [128, n/16]


def build_idx_tensors(layout, percore):
    """Per-core DRAM tensors: xlidx/xridx [128, TOTI] i16 (call-major,
    wrapped per call), reld [128, TOTC] bf16 (slot-major)."""
    calls = layout['calls']
    ncols_g = layout['ncols_g']
    gc0 = layout['gc0']
    TOTC = int(ncols_g.sum())
    TOTI = sum(k for (_, _, _, k, _) in calls) // 16
    out = []
    for c in range(NCORES):
        xl_idx, xr_idx, reld = percore[c]            # [TOTC, P] slot-major
        xl_flat = np.zeros((P, TOTI), np.int16)
        xr_flat = np.zeros((P, TOTI), np.int16)
        io = 0
        for (g, w, off, k, ocb) in calls:
            c0 = gc0[g] + ocb                         # global out col base
            ncol = k // P
            xlv = xl_idx[c0:c0 + ncol].reshape(-1)    # [k] slot order
            xrv = xr_idx[c0:c0 + ncol].reshape(-1)
            xl_flat[:, io:io + k // 16] = wrap_i16(xlv)
            xr_flat[:, io:io + k // 16] = wrap_i16(xrv)
            io += k // 16
        reldt = np.ascontiguousarray(reld.T.astype(BF16))   # [P, TOTC]
        out.append((xl_flat, xr_flat, reldt))
    return out, TOTC, TOTI


# ---------------------------------------------------------------- bass build

def build_program(layout, TOTC, TOTI, debug=False):
    import concourse.bass as bass
    import concourse.mybir as mybir
    from concourse import bacc, library_config
    from concourse.tile import TileContext

    f32 = mybir.dt.float32
    i16 = mybir.dt.int16
    bf16 = mybir.dt.bfloat16
    AF = mybir.ActivationFunctionType
    OP = mybir.AluOpType

    calls = layout['calls']
    ncols_g = layout['ncols_g']
    gc0 = layout['gc0']
    blk_of_col = layout['blk_of_col']
    dims_in = [F_IN, D, D]

    nc = bacc.Bacc()
    x0 = nc.declare_dram_parameter("x0", [NSHARD, F_IN], bf16, isOutput=False)
    xlidx = nc.declare_dram_parameter("xlidx", [P, TOTI], i16, isOutput=False)
    xridx = nc.declare_dram_parameter("xridx", [P, TOTI], i16, isOutput=False)
    reldp = nc.declare_dram_parameter("reld", [P, TOTC], bf16, isOutput=False)
    wp = []
    for l in range(L):
        din = dims_in[l]
        wp.append((
            nc.declare_dram_parameter(f"Wl{l}", [din, D], bf16, isOutput=False),
            nc.declare_dram_parameter(f"Wr{l}", [din, D], bf16, isOutput=False),
            nc.declare_dram_parameter(f"attb{l}", [P, D], bf16, isOutput=False),
            nc.declare_dram_parameter(f"biasb{l}", [P, D], f32, isOutput=False),
        ))
    ident_in = nc.declare_dram_parameter("ident", [P, P], bf16, isOutput=False)
    iota_in = nc.declare_dram_parameter("iota", [P, P], bf16, isOutput=False)
    y = nc.declare_dram_parameter("y", [NGRP * G * P, D], f32, isOutput=True)
    if debug:
        dbg_xl = nc.declare_dram_parameter("dbg_xl", [NP, P], bf16, isOutput=True)
        dbg_xr = nc.declare_dram_parameter("dbg_xr", [NSHARD, D], bf16, isOutput=True)
        dbg_y0 = nc.declare_dram_parameter("dbg_y0", [NGRP * G * P, D], bf16, isOutput=True)
        c0g = int(layout['ncols_g'][0])
        dbg_xls = nc.declare_dram_parameter("dbg_xls", [P, c0g * P], bf16, isOutput=True)
        dbg_vals = nc.declare_dram_parameter("dbg_vals", [P, c0g * W], bf16, isOutput=True)
        dbg_ot = nc.declare_dram_parameter("dbg_ot", [P, c0g * P], bf16, isOutput=True)

    # internal DRAM (ping-pong across layers to avoid WAR on gathers)
    xl_sh = [nc.dram_tensor(f"xl_sh{i}", [NSHARD, P], bf16) for i in range(2)]
    xl_full = [nc.dram_tensor(f"xl_full{i}", [NP, P], bf16, addr_space="Shared")
               for i in range(2)]
    xr_tab = [nc.dram_tensor(f"xr_tab{i}", [NSHARD, D], bf16) for i in range(2)]
    xmid = [nc.dram_tensor(f"xmid{i}", [NGRP * G * P, D], bf16) for i in range(2)]

    with TileContext(nc) as tc:
        with (
            tc.tile_pool(name="const", bufs=1) as cp,
            tc.tile_pool(name="tf", bufs=3) as tfp,
            tc.tile_pool(name="tfps", bufs=1, space="PSUM") as tfps,
            tc.tile_pool(name="eidx", bufs=2) as eip,
            tc.tile_pool(name="edge", bufs=2) as ep,
            tc.tile_pool(name="eps", bufs=1, space="PSUM") as eps,
            tc.tile_pool(name="fin", bufs=2) as fp_,
        ):
            nc.gpsimd.load_library(library_config.mlp)
            ident = cp.tile([P, P], bf16, tag="ident")
            nc.sync.dma_start(out=ident[:], in_=ident_in[:])
            iota = cp.tile([P, P], bf16, tag="iota")
            nc.sync.dma_start(out=iota[:], in_=iota_in[:])
            wts = []
            for l in range(L):
                din = dims_in[l]
                wl = cp.tile([din, D], bf16, tag=f"wl{l}")
                nc.sync.dma_start(out=wl[:], in_=wp[l][0][:])
                wr = cp.tile([din, D], bf16, tag=f"wr{l}")
                nc.sync.dma_start(out=wr[:], in_=wp[l][1][:])
                attb = cp.tile([P, D], bf16, tag=f"attb{l}")
                nc.sync.dma_start(out=attb[:], in_=wp[l][2][:])
                biasb = cp.tile([P, D], f32, tag=f"biasb{l}")
                nc.sync.dma_start(out=biasb[:], in_=wp[l][3][:])
                wts.append((wl, wr, attb, biasb))

            for l in range(L):
                din = dims_in[l]
                wl, wr, attb, biasb = wts[l]
                xlS, xlF, xrT = xl_sh[l % 2], xl_full[l % 2], xr_tab[l % 2]
                x_cur = x0 if l == 0 else xmid[(l - 1) % 2]
                x_out = y if l == L - 1 else xmid[l % 2]

                # ---- node transform (own shard, groups of 2 blocks)
                for t0 in range(0, NBLK, 2):
                    gsz = min(2, NBLK - t0)
                    rb = t0 * P
                    xc = tfp.tile([P, gsz * din], bf16, tag="xc")
                    nc.sync.dma_start(
                        out=xc[:].rearrange("p (g d) -> p g d", g=gsz),
                        in_=x_cur[rb:rb + gsz * P, 0:din].rearrange(
                            "(g p) d -> p g d", p=P))
                    xt_ps = tfps.tile([din, gsz * P], bf16, tag="xtps")
                    for k in range(gsz):
                        nc.tensor.transpose(
                            out=xt_ps[:, k * P:(k + 1) * P],
                            in_=xc[:, k * din:(k + 1) * din],
                            identity=ident[:])
                    xt = tfp.tile([din, gsz * P], bf16, tag="xt")
                    nc.scalar.activation(out=xt[:], in_=xt_ps[:], func=AF.Copy)
                    mm = tfps.tile([P, gsz * 2 * D], f32, tag="mm")
                    for k in range(gsz):
                        nc.tensor.matmul(
                            out=mm[:, (2 * k) * D:(2 * k + 1) * D],
                            lhsT=xt[:, k * P:(k + 1) * P], rhs=wl[:],
                            start=True, stop=True)
                        nc.tensor.matmul(
                            out=mm[:, (2 * k + 1) * D:(2 * k + 2) * D],
                            lhsT=xt[:, k * P:(k + 1) * P], rhs=wr[:],
                            start=True, stop=True)
                    xlo = tfp.tile([P, gsz * P], bf16, tag="xlo")
                    xro = tfp.tile([P, gsz * D], bf16, tag="xro")
                    mm_v = mm[:].rearrange("p (g two d) -> p g two d",
                                           g=gsz, two=2)
                    nc.scalar.activation(
                        out=xlo[:].rearrange("p (g d) -> p g d", g=gsz)[:, :, 0:D],
                        in_=mm_v[:, :, 0, :],
                        func=AF.Copy)
                    nc.vector.tensor_copy(
                        out=xro[:].rearrange("p (g d) -> p g d", g=gsz),
                        in_=mm_v[:, :, 1, :])
                    nc.sync.dma_start(
                        out=xlS[rb:rb + gsz * P, :].rearrange(
                            "(g p) d -> p g d", p=P),
                        in_=xlo[:].rearrange("p (g d) -> p g d", g=gsz))
                    nc.sync.dma_start(
                        out=xrT[rb:rb + gsz * P, :].rearrange(
                            "(g p) d -> p g d", p=P),
                        in_=xro[:].rearrange("p (g d) -> p g d", g=gsz))

                # ---- AllGather xl (padded bf16 rows)
                nc.gpsimd.collective_compute(
                    "AllGather", OP.bypass,
                    replica_groups=[list(range(NCORES))],
                    ins=[xlS[:]], outs=[xlF[:]],
                )

                if debug and l == 0:
                    nc.sync.dma_start(out=dbg_xl[:], in_=xlF[:])
                    nc.sync.dma_start(out=dbg_xr[:], in_=xrT[:])

                # ---- edge phase (25 groups of 4 blocks)
                gcalls = {}
                io = 0
                for (g_, w_, off_, k_, ocb_) in calls:
                    gcalls.setdefault(g_, []).append((w_, off_, k_, ocb_, io))
                    io += k_ // 16
                for g in range(NGRP):
                    cols = int(ncols_g[g])
                    c0 = int(gc0[g])
                    boc = blk_of_col[g]
                    # idx/reld tiles
                    myc = gcalls.get(g, [])
                    i0 = myc[0][4]
                    itot = sum(k_ // 16 for (_, _, k_, _, _) in myc)
                    xli = eip.tile([P, itot], i16, tag="xli")
                    nc.sync.dma_start(out=xli[:], in_=xlidx[:, i0:i0 + itot])
                    reld = eip.tile([P, cols], bf16, tag="reld")
                    nc.sync.dma_start(out=reld[:], in_=reldp[:, c0:c0 + cols])
                    # one-hot (edge-major) — needed early for the xr broadcast
                    ot = ep.tile([P, cols * P], bf16, tag="ot")
                    nc.vector.tensor_tensor(
                        out=ot[:].rearrange("p (c q) -> p c q", c=cols),
                        in0=reld[:].unsqueeze(2).to_broadcast([P, cols, P]),
                        in1=iota[:].unsqueeze(1).to_broadcast([P, cols, P]),
                        op=OP.is_equal)
                    # xl gather (Pool/SWDGE)
                    xl_s = ep.tile([P, cols * P], bf16, tag="xls")
                    xl_v = xl_s[:].rearrange("p (c e) -> p c e", c=cols)
                    for (w_, off_, k_, ocb_, io_) in myc:
                        nc.gpsimd.dma_gather(
                            out_ap=xl_v[:, ocb_:ocb_ + k_ // P, :],
                            in_ap=xlF[w_ * WIN:(w_ + 1) * WIN, :],
                            idxs_ap=xli[:, io_ - i0:io_ - i0 + k_ // 16],
                            num_idxs=k_, num_idxs_reg=k_, elem_size=P,
                            single_packet=False)
                    # xr per edge via PE broadcast: otT.T @ xr_blk (no gather)
                    nb = min(G, NBLK - g * G)       # real blocks in group
                    xr_blkt = ep.tile([P, G * D], bf16, tag="xrblk")
                    nc.sync.dma_start(
                        out=xr_blkt[:, 0:nb * D].rearrange(
                            "p (b d) -> p b d", b=nb),
                        in_=xrT[g * G * P:g * G * P + nb * P, :].rearrange(
                            "(b p) d -> p b d", p=P))
                    xr_e = ep.tile([P, cols * D], bf16, tag="xre")
                    otT_ps = eps.tile([P, 2 * P], bf16, tag="otTps")
                    otT_sb = ep.tile([P, 2 * P], bf16, tag="otTsb")
                    xre_ps = eps.tile([P, 4 * D], f32, tag="xreps")
                    for j in range(cols):
                        b = int(boc[j])
                        s2 = (j % 2) * P
                        s4 = (j % 4) * D
                        nc.tensor.transpose(
                            out=otT_ps[:, s2:s2 + P],
                            in_=ot[:, j * P:(j + 1) * P],
                            identity=ident[:])
                        nc.scalar.activation(
                            out=otT_sb[:, s2:s2 + P],
                            in_=otT_ps[:, s2:s2 + P], func=AF.Copy)
                        nc.tensor.matmul(
                            out=xre_ps[:, s4:s4 + D],
                            lhsT=otT_sb[:, s2:s2 + P],
                            rhs=xr_blkt[:, b * D:(b + 1) * D],
                            start=True, stop=True)
                        nc.scalar.activation(
                            out=xr_e[:, j * D:(j + 1) * D],
                            in_=xre_ps[:, s4:s4 + D], func=AF.Copy)
                    xr_v = xr_e[:].rearrange("p (c e) -> p c e", c=cols)
                    # scores: g = leaky(xl+xr); u = g*att; e = sum_c u
                    nc.vector.tensor_tensor(
                        out=xr_v[:], in0=xl_v[:, :, 0:D],
                        in1=xr_v[:], op=OP.add)
                    nc.scalar.activation(
                        out=xr_e[:], in_=xr_e[:],
                        func=AF.Prelu, alpha=NEG_SLOPE)
                    nc.vector.tensor_tensor(
                        out=xr_v[:], in0=xr_v[:],
                        in1=attb[:].unsqueeze(1).to_broadcast([P, cols, D]),
                        op=OP.mult)
                    e_t = ep.tile([P, cols * H], f32, tag="e")
                    nc.vector.tensor_reduce(
                        out=e_t[:].rearrange("p (c h) -> p c h", c=cols),
                        in_=xr_e[:].rearrange("p (c h k) -> p c h k",
                                              c=cols, h=H),
                        axis=mybir.AxisListType.X, op=OP.add)
                    vals = ep.tile([P, cols * W], bf16, tag="vals")
                    vals_v = vals[:].rearrange("p (c w) -> p c w", c=cols)
                    nc.scalar.activation(
                        out=vals_v[:, :, D:W],
                        in_=e_t[:].rearrange("p (c h) -> p c h", c=cols),
                        func=AF.Exp)
                    nc.vector.tensor_tensor(
                        out=vals_v[:, :, 0:D].rearrange(
                            "p c (h k) -> p c h k", h=H),
                        in0=xl_v[:, :, 0:D].rearrange(
                            "p c (h k) -> p c h k", h=H),
                        in1=vals_v[:, :, D:W].unsqueeze(3).to_broadcast(
                            [P, cols, H, C]),
                        op=OP.mult)
                    if debug and l == 0 and g == 0:
                        nc.sync.dma_start(out=dbg_xls[:], in_=xl_s[:])
                        nc.sync.dma_start(out=dbg_vals[:], in_=vals[:])
                        nc.sync.dma_start(out=dbg_ot[:], in_=ot[:])
                    # aggregation — one PSUM tile (bank) per dst block:
                    # interleaved accumulation regions within one bank are
                    # broken on HW (later start=True clobbers earlier
                    # regions' partials)
                    psb = []
                    for b_ in range(G):
                        t_ = eps.tile([P, W], f32, tag=f"ps{b_}",
                                      name=f"ps{b_}_{l}_{g}")
                        psb.append(t_)
                    first = [True] * G
                    lastcol = {}
                    for j in range(cols):
                        lastcol[int(boc[j])] = j
                    for b_ in range(G):
                        if b_ not in lastcol:      # pad block: no edges
                            nc.vector.memset(psb[b_][:], 0.0)
                    for j in range(cols):
                        b = int(boc[j])
                        nc.tensor.matmul(
                            out=psb[b][:],
                            lhsT=ot[:, j * P:(j + 1) * P],
                            rhs=vals[:, j * W:(j + 1) * W],
                            start=first[b], stop=(lastcol[b] == j))
                        first[b] = False
                    # finalize
                    den = fp_.tile([P, G * H], f32, tag="den")
                    for b_ in range(G):
                        nc.vector.tensor_scalar(
                            out=den[:, b_ * H:(b_ + 1) * H],
                            in0=psb[b_][:, D:W], scalar1=1e-30, scalar2=None,
                            op0=OP.max)
                    r = fp_.tile([P, G * H], f32, tag="r")
                    nc.vector.reciprocal(out=r[:], in_=den[:])
                    o = fp_.tile([P, G * D], f32, tag="o")
                    o_v = o[:].rearrange("p (g d) -> p g d", g=G)
                    for b_ in range(G):
                        nc.vector.tensor_tensor(
                            out=o[:, b_ * D:(b_ + 1) * D].rearrange(
                                "p (h k) -> p h k", h=H),
                            in0=psb[b_][:, 0:D].rearrange(
                                "p (h k) -> p h k", h=H),
                            in1=r[:, b_ * H:(b_ + 1) * H].unsqueeze(2)
                                .to_broadcast([P, H, C]),
                            op=OP.mult)
                    nc.vector.tensor_tensor(
                        out=o_v[:], in0=o_v[:],
                        in1=biasb[:].unsqueeze(1).to_broadcast([P, G, D]),
                        op=OP.add)
                    t1 = fp_.tile([P, G * D], f32, tag="t1")
                    nc.vector.tensor_scalar(
                        out=t1[:], in0=o[:], scalar1=0.0, scalar2=None,
                        op0=OP.min)
                    nc.scalar.activation(out=t1[:], in_=t1[:], func=AF.Exp)
                    yo = fp_.tile([P, G * D], f32 if l == L - 1 else bf16,
                                  tag="yo")
                    nc.vector.scalar_tensor_tensor(
                        out=yo[:], in0=t1[:], scalar=-1.0, in1=o[:],
                        op0=OP.add, op1=OP.max)
                    nc.sync.dma_start(
                        out=x_out[g * G * P:(g + 1) * G * P, :].rearrange(
                            "(g p) d -> p g d", p=P),
                        in_=yo[:].rearrange("p (g d) -> p g d", g=G))
                if debug and l == 0:
                    nc.sync.dma_start(out=dbg_y0[:], in_=xmid[0][:])
    nc.compile()
    return nc


# ---------------------------------------------------------------- entry

def make_inmaps(inputs):
    x = np.asarray(inputs['x'], np.float32)
    ei = np.asarray(inputs['edge_index'], np.int32)
    loops = np.arange(N_NODES, dtype=np.int64)
    src = np.concatenate([ei[0].astype(np.int64), loops])
    dst = np.concatenate([ei[1].astype(np.int64), loops])
    layout, percore = prep_edges(src, dst)
    idx_tensors, TOTC, TOTI = build_idx_tensors(layout, percore)

    xp = np.zeros((NP, F_IN), np.float32)
    xp[:N_NODES] = x
    xp16 = xp.astype(BF16)
    iota = np.broadcast_to(np.arange(P, dtype=np.float32), (P, P)).astype(BF16)
    ident = np.eye(P, dtype=np.float32).astype(BF16)
    base = {"ident": np.ascontiguousarray(ident),
            "iota": np.ascontiguousarray(iota)}
    for l in range(L):
        wlv = np.asarray(inputs[f'Wl{l}'], np.float32).astype(BF16)
        wrv = np.asarray(inputs[f'Wr{l}'], np.float32).astype(BF16)
        att = np.asarray(inputs[f'att{l}'], np.float32).reshape(1, D)
        b = np.asarray(inputs[f'b{l}'], np.float32).reshape(1, D)
        base[f"Wl{l}"] = np.ascontiguousarray(wlv)
        base[f"Wr{l}"] = np.ascontiguousarray(wrv)
        base[f"attb{l}"] = np.ascontiguousarray(
            np.broadcast_to(att, (P, D)).astype(BF16))
        base[f"biasb{l}"] = np.ascontiguousarray(
            np.broadcast_to(b, (P, D)).astype(np.float32))
    in_maps = []
    for c in range(NCORES):
        xl_flat, xr_flat, reldt = idx_tensors[c]
        m = dict(base)
        m["x0"] = np.ascontiguousarray(xp16[c * NSHARD:(c + 1) * NSHARD])
        m["xlidx"] = np.ascontiguousarray(xl_flat)
        m["xridx"] = np.ascontiguousarray(xr_flat)
        m["reld"] = reldt
        in_maps.append(m)
    return in_maps, layout, TOTC, TOTI


def kernel(**inputs):
    from concourse.bass_utils import run_bass_kernel_spmd
    in_maps, layout, TOTC, TOTI = make_inmaps(inputs)
    nc = build_program(layout, TOTC, TOTI)
    res = run_bass_kernel_spmd(nc, in_maps, list(range(NCORES)))
    out = np.concatenate(
        [np.asarray(res.results[c]["y"])[:NSHARD] for c in range(NCORES)],
        axis=0)
    return out[:N_NODES].astype(np.float32)


if __name__ == "__main__":
    pass
